# revision 27
# baseline (speedup 1.0000x reference)
"""Trainium2 Bass kernel for pre-LN multi-head attention.

Reference computation (B=2, N=2048, D=1024, H=16, DH=64):
    xn = LayerNorm(x) * g + b
    q = xn @ Wq ; k, v = split(xn @ Wkv)
    out = softmax(q k^T / sqrt(DH)) v  (per head)
    return out @ Wout
Sharding: core c handles batch b = c // 4 and heads 4*(c%4) .. 4*(c%4)+3.
Each core computes a partial output; the host sums 4 partials per batch.

Key performance structure (v2):
- x and all weights ship from the host in bf16 (halves input DMA).
- Attention processes heads in PAIRS that share the j index but differ in
  base partition (0 vs 64). The two heads' score matmuls (contraction
  DH=64) land on different PE row-groups and run CONCURRENTLY via the
  tile_position auto-derivation (row tiling), halving score cost.
- Softmax exp is split across engines: head A of each pair uses the exact
  Scalar-engine Exp; head B uses a fast exp on the Vector engine
  (Schraudolph: I = trunc(a*s + b) as int16, bitcast to bf16). The
  ~1.8% multiplicative noise on B-tiles is tolerable (rel gate 2e-2) and
  partially cancels via the shared rowsum normalization.
- Attention loop nest is (head-pair, qc, kt) with qc outer so the score
  PSUM tiles are [128, 512]: 2 heads x 2 buffers (4 banks) + 2
  accumulator banks [65, 512] fit the 8 PSUM banks with double buffering.
- Rowsum comes from a ones-column appended to V; reciprocal is
  partition-broadcast via a DRAM bounce.
"""

from contextlib import ExitStack

import numpy as np

import concourse.bass as bass
import concourse.mybir as mybir
import concourse.tile as tile
from concourse import bacc
from concourse.bass_utils import run_bass_kernel_spmd

import ml_dtypes

B, N, D = 2, 2048, 1024
H, DH = 16, 64
EPS = 1e-5
N_CORES = 8
HPC = 4          # heads per core
CW = HPC * DH    # 256 columns of q/k/v per core

f32 = mybir.dt.float32
bf16 = mybir.dt.bfloat16
i16 = mybir.dt.int16
AF = mybir.ActivationFunctionType
ALU = mybir.AluOpType

# fast-exp constants: I = trunc(A_EXP * s + B_EXP) as int16 -> bf16 bits
A_EXP = 128.0 * 1.4426950408889634     # 128 * log2(e)
B_EXP = 128.0 * (127.0 - 0.0434609) + 0.5

TRACE = False
LAST_RESULT = None
_compiled = None


def _build():
    nc = bacc.Bacc("TRN2", target_bir_lowering=False, debug=False,
                   num_devices=N_CORES)

    x_d = nc.dram_tensor("x", [N, D], bf16, kind="ExternalInput")
    wq_d = nc.dram_tensor("wq", [D, CW], bf16, kind="ExternalInput")
    wk_d = nc.dram_tensor("wk", [D, CW], bf16, kind="ExternalInput")
    wv_d = nc.dram_tensor("wv", [D, CW], bf16, kind="ExternalInput")
    wo_d = nc.dram_tensor("wo", [CW, D], bf16, kind="ExternalInput")
    cq_d = nc.dram_tensor("cq", [CW], f32, kind="ExternalInput")
    ck_d = nc.dram_tensor("ck", [CW], f32, kind="ExternalInput")
    cv_d = nc.dram_tensor("cv", [CW], f32, kind="ExternalInput")
    out_d = nc.dram_tensor("out", [N, D], f32, kind="ExternalOutput")
    rec_d = nc.dram_tensor("rec_scratch", [16, 512], f32)
    sum_d = nc.dram_tensor("sum_scratch", [16, 512], f32)
    ident_d = nc.dram_tensor("ident", [128, 128], bf16, kind="ExternalInput")

    with tile.TileContext(nc) as tc, ExitStack() as ctx:
        consts = ctx.enter_context(tc.tile_pool(name="consts", bufs=1))
        zTp = ctx.enter_context(tc.tile_pool(name="zTp", bufs=1))

        ident = consts.tile([128, 128], bf16)
        nc.sync.dma_start(out=ident, in_=ident_d[:])

        eps_t = consts.tile([128, 1], f32)
        nc.vector.memset(eps_t, EPS)

        # x prefetch first so its DMAs lead the queue; weights follow.
        # (stays open for the whole kernel; SBUF budget allows it)
        xpre = ctx.enter_context(tc.tile_pool(name="xpre", bufs=8))
        x_r = x_d.rearrange("(t u p) d -> t p u d", p=128, u=2)
        x_tiles = []
        for rt in range(8):
            xt = xpre.tile([128, 2, D], bf16, tag="xt")
            nc.sync.dma_start(out=xt, in_=x_r[rt])
            x_tiles.append(xt)

        # weights arrive bf16 — direct DMA, no staging cast
        wq_t = consts.tile([128, 8, CW], bf16)
        wk_t = consts.tile([128, 8, CW], bf16)
        wv_t = consts.tile([128, 8, CW], bf16)
        wo_t = consts.tile([128, 2, D], bf16)
        for dram, dst, spec in ((wq_d, wq_t, "(t p) m -> p t m"),
                               (wk_d, wk_t, "(t p) m -> p t m"),
                               (wv_d, wv_t, "(t p) m -> p t m"),
                               (wo_d, wo_t, "(j p) d -> p j d")):
            nc.sync.dma_start(out=dst, in_=dram.rearrange(spec, p=128))

        cq_t = consts.tile([128, 2], f32)
        nc.sync.dma_start(out=cq_t, in_=cq_d.rearrange("(j p) -> p j", p=128))
        ck_t = consts.tile([128, 2], f32)
        nc.sync.dma_start(out=ck_t, in_=ck_d.rearrange("(j p) -> p j", p=128))
        cv_t = consts.tile([128, 2], f32)
        nc.sync.dma_start(out=cv_t, in_=cv_d.rearrange("(j p) -> p j", p=128))

        # z^T in bf16: [chan(128 per tile) x 8 chan-tiles x N rows]
        zT = zTp.tile([128, 8, N], bf16)

        # dummy matmuls during the input-DMA wait: trips the HAM activity
        # window so the real matmul stream starts at the full 2.4 GHz clock
        with tc.tile_pool(name="psW", bufs=1, space="PSUM") as psW:
            pw = psW.tile([128, 128], f32)
            for i in range(40):
                nc.tensor.matmul(pw, ident, ident,
                                 start=(i == 0), stop=(i == 39))

        # ---- stage 1 + 2 interleaved: LayerNorm/transpose into zT, with
        # QKV chunk-pairs emitted as soon as their zT columns are ready so
        # the PE chews on projections while DVE/ACT run LN for later rows.
        with tc.tile_pool(name="qkT", bufs=1) as qkTp, \
             tc.tile_pool(name="vaug", bufs=1) as vaugp:
            qT = qkTp.tile([128, 2, N], bf16, tag="qT")
            kT = qkTp.tile([128, 2, N], bf16, tag="kT")
            vT = qkTp.tile([128, 2, N], bf16, tag="vT")
            vA = vaugp.tile([128, 16, HPC * (DH + 1)], bf16)
            vA4 = vA.rearrange("p k (h c) -> p k h c", h=HPC)
            ones64 = vaugp.tile([128, 64], f32)
            nc.vector.memset(ones64, 1.0)
            nc.vector.tensor_copy(
                out=vA4[:, :, :, DH:DH + 1],
                in_=ones64.rearrange("p (k h w) -> p k h w", k=16, h=HPC, w=1))

            with tc.tile_pool(name="zp", bufs=4) as zp, \
                 tc.tile_pool(name="stp", bufs=12) as stp, \
                 tc.tile_pool(name="ps1", bufs=2, space="PSUM") as ps1, \
                 tc.tile_pool(name="ps2", bufs=2, space="PSUM") as ps2:

                def ln_rt(rt):
                    xt = x_tiles[rt]
                    zt = zp.tile([128, 2, D], bf16)
                    scales = []
                    for u in range(2):
                        st = stp.tile([128, 2, 6], f32, name=f"st{rt}_{u}")
                        nc.vector.bn_stats(out=st[:, 0], in_=xt[:, u, 0:512])
                        nc.vector.bn_stats(out=st[:, 1],
                                           in_=xt[:, u, 512:1024])
                        mv = stp.tile([128, 2], f32, name=f"mv{rt}_{u}")
                        nc.vector.bn_aggr(out=mv, in_=st)
                        rstd = stp.tile([128, 1], f32, name=f"rstd{rt}_{u}")
                        nc.scalar.activation(out=rstd, in_=mv[:, 1:2],
                                             func=AF.Sqrt, bias=eps_t,
                                             scale=1.0)
                        nc.vector.reciprocal(out=rstd, in_=rstd)
                        nmr = stp.tile([128, 1], f32, name=f"nmr{rt}_{u}")
                        nc.vector.tensor_scalar(out=nmr, in0=mv[:, 0:1],
                                                scalar1=rstd, scalar2=-1.0,
                                                op0=ALU.mult, op1=ALU.mult)
                        scales.append((rstd, nmr))
                    for u in range(2):
                        rstd, nmr = scales[u]
                        for half in range(2):
                            hs = slice(half * 512, (half + 1) * 512)
                            nc.scalar.activation(out=zt[:, u, hs],
                                                 in_=xt[:, u, hs],
                                                 func=AF.Identity, bias=nmr,
                                                 scale=rstd)
                        r0 = rt * 256 + u * 128
                        pt = ps1.tile([128, 1024], bf16)
                        for cc in range(8):
                            nc.tensor.transpose(
                                pt[:, cc * 128:(cc + 1) * 128],
                                zt[:, u, cc * 128:(cc + 1) * 128], ident)
                        cp_out = zT[:, :, r0:r0 + 128]
                        cp_in = pt.rearrange("p (c n) -> p c n", c=8)
                        if u == 0:
                            nc.vector.tensor_copy(out=cp_out, in_=cp_in)
                        else:
                            nc.scalar.activation(out=cp_out, in_=cp_in,
                                                 func=AF.Identity, bias=0.0,
                                                 scale=1.0)

                def qkv_chunkpair(cp):
                    chks = (2 * cp, 2 * cp + 1)
                    for w_t, c_t, dest in ((wq_t, cq_t, qT), (wk_t, ck_t, kT),
                                           (wv_t, cv_t, vT)):
                        for j in range(2):
                            pqs = {c: ps2.tile([128, 512], f32,
                                               tag=f"pq{c % 2}",
                                               name=f"pq{cp}_{c}")
                                   for c in chks}
                            for t in range(8):
                                lhs = w_t[:, t, j * 128:(j + 1) * 128]
                                for chk in chks:
                                    nc.tensor.matmul(
                                        pqs[chk], lhs,
                                        zT[:, t, chk * 512:(chk + 1) * 512],
                                        start=(t == 0), stop=(t == 7))
                            for chk in chks:
                                ns = slice(chk * 512, (chk + 1) * 512)
                                nc.vector.tensor_scalar_add(
                                    out=dest[:, j, ns], in0=pqs[chk],
                                    scalar1=c_t[:, j:j + 1])

                ln_rt(0), ln_rt(1), ln_rt(2), ln_rt(3)
                qkv_chunkpair(0)
                ln_rt(4), ln_rt(5), ln_rt(6), ln_rt(7)
                qkv_chunkpair(1)

                # transpose vT -> vA[keys, c] (heads 2j, 2j+1 per j-half)
                with tc.tile_pool(name="psVT", bufs=1, space="PSUM") as psVT:
                    for kt in range(16):
                        ks = slice(kt * 128, (kt + 1) * 128)
                        for j in range(2):
                            pt = psVT.tile([128, 128], bf16, tag=f"vt{j}",
                                           name=f"vt{kt}_{j}")
                            nc.tensor.transpose(pt, vT[:, j, ks], ident)
                            nc.vector.tensor_copy(
                                out=vA4[:, kt, 2 * j:2 * j + 2, 0:DH],
                                in_=pt.rearrange("p (h c) -> p h c", h=2))

            # ---- stage 3: attention, head pairs x qc pairs ----
            # Per (hp, qcp, kt): the two heads' score MMs go to PE row-groups
            # 0/64 and run concurrently; each lhsT serves the two qc chunks.
            # exp is split: ACT does head A + 1/4 of head B (exact), DVE the
            # other 3/4 of head B (fast int16-bitcast exp).
            with tc.tile_pool(name="oT", bufs=1) as oTp:
                outT = oTp.tile([128, 2, N], bf16)
                with tc.tile_pool(name="Ep", bufs=4) as Ep, \
                     tc.tile_pool(name="rp", bufs=6) as rp, \
                     tc.tile_pool(name="osb", bufs=4) as osb, \
                     tc.tile_pool(name="psD", bufs=1, space="PSUM") as psD, \
                     tc.tile_pool(name="psU", bufs=1, space="PSUM") as psU:
                    out_r = out_d.rearrange("(m p) d -> m p d", p=128)

                    def outproj_half(qcp):
                        # m-tiles 8*qcp .. 8*qcp+7 use outT cols that are
                        # final once both head-pairs finished this qcp;
                        # PSUM comes from the psD ring (same tile shape)
                        for m in range(8 * qcp, 8 * qcp + 8):
                            pOs = [psD.tile([128, 512], f32,
                                            tag=f"pD{i}_0",
                                            name=f"pO{m}_{i}")
                                   for i in range(2)]
                            for j in range(2):
                                lhs = outT[:, j, m * 128:(m + 1) * 128]
                                for nn in range(2):
                                    nc.tensor.matmul(
                                        pOs[nn], lhs,
                                        wo_t[:, j, nn * 512:(nn + 1) * 512],
                                        start=(j == 0), stop=(j == 1))
                            for nn in range(2):
                                ot = osb.tile([128, 512], f32)
                                if nn == 0:
                                    nc.vector.tensor_copy(out=ot, in_=pOs[nn])
                                else:
                                    nc.scalar.activation(out=ot, in_=pOs[nn],
                                                         func=AF.Identity,
                                                         bias=0.0, scale=1.0)
                                nc.sync.dma_start(
                                    out=out_r[m][:, nn * 512:(nn + 1) * 512],
                                    in_=ot)

                    for qcp in range(2):         # query column pair (2x512)
                        for hp in range(2):      # head pair: heads 2hp, 2hp+1
                            pUs = [[psU.tile([DH + 1, 512], f32,
                                             tag=f"pU{hi}_{q2}",
                                             name=f"pU{hp}_{qcp}_{hi}_{q2}")
                                    for q2 in range(2)] for hi in range(2)]
                            Ets = {}

                            def scores_exp(kt, hp=hp, qcp=qcp, pUs=pUs,
                                           Ets=Ets):
                                ks = slice(kt * 128, (kt + 1) * 128)
                                pDs = [[psD.tile([128, 512], f32,
                                                 tag=f"pD{hi}_{q2}",
                                                 name=f"pD{hp}{qcp}{kt}{hi}{q2}")
                                        for q2 in range(2)]
                                       for hi in range(2)]
                                # A0 A1 then B0 B1: each kT lhsT serves two
                                # MMs; B's row-group differs so it overlaps A
                                for hi in range(2):
                                    p0 = 64 * hi
                                    for q2 in range(2):
                                        qs = slice((qcp * 2 + q2) * 512,
                                                   (qcp * 2 + q2 + 1) * 512)
                                        nc.tensor.matmul(
                                            pDs[hi][q2],
                                            kT[p0:p0 + 64, hp, ks],
                                            qT[p0:p0 + 64, hp, qs],
                                            start=True, stop=True)
                                # exp: one whole-tile op per pD tag so each
                                # tag frees after a single engine pass —
                                # head A on ACT (exact), head B on DVE (fast)
                                EtA = Ep.tile([128, 2, 512], bf16, tag="EtA")
                                EtB = Ep.tile([128, 2, 512], i16, tag="EtB")
                                EtBf = EtB.bitcast(bf16)
                                nc.vector.tensor_scalar(
                                    out=EtB[:, 0], in0=pDs[1][0],
                                    scalar1=A_EXP, scalar2=B_EXP,
                                    op0=ALU.mult, op1=ALU.add)
                                for q2 in range(2):
                                    nc.scalar.activation(
                                        out=EtA[:, q2], in_=pDs[0][q2],
                                        func=AF.Exp, bias=0.0, scale=1.0)
                                nc.scalar.activation(
                                    out=EtBf[:, 1], in_=pDs[1][1],
                                    func=AF.Exp, bias=0.0, scale=1.0)
                                Ets[kt] = (EtA, EtBf)

                            def ev(kt, hp=hp, pUs=pUs, Ets=Ets):
                                EtA, EtBf = Ets.pop(kt)
                                for hi, Et in ((0, EtA), (1, EtBf)):
                                    h = 2 * hp + hi
                                    vlhs = vA[:, kt,
                                              h * (DH + 1):(h + 1) * (DH + 1)]
                                    for q2 in range(2):
                                        nc.tensor.matmul(
                                            pUs[hi][q2][0:DH + 1, :],
                                            vlhs, Et[:, q2],
                                            start=(kt == 0), stop=(kt == 15))

                            # software-pipelined emission: EV trails the
                            # scores/exp of the NEXT kt so the in-order PE
                            # queue never head-blocks on exp or on the
                            # previous block's normalization
                            scores_exp(0)
                            scores_exp(1)
                            for kt in range(2, 16):
                                ev(kt - 2)
                                scores_exp(kt)
                            ev(14)
                            ev(15)
                            # normalization: rowsum -> DRAM bounce -> recip
                            for hi in range(2):
                                for q2 in range(2):
                                    pU = pUs[hi][q2]
                                    qc = qcp * 2 + q2
                                    qs = slice(qc * 512, (qc + 1) * 512)
                                    p0 = 64 * hi
                                    slot_id = hp * 8 + qc * 2 + hi
                                    # full copy frees the PSUM accumulator
                                    # immediately; norm runs off the copy
                                    uS = rp.tile([128, 512], f32, tag="uS",
                                                 name=f"uS{slot_id}")
                                    nc.vector.tensor_copy(
                                        out=uS[0:DH + 1, :],
                                        in_=pU[0:DH + 1, :])
                                    sslot = sum_d[slot_id]
                                    nc.sync.dma_start(out=sslot,
                                                      in_=uS[DH:DH + 1, :])
                                    r8 = rp.tile([64, 8], f32, tag="r8",
                                                 name=f"r8{slot_id}")
                                    nc.sync.dma_start(
                                        out=r8,
                                        in_=sslot.rearrange("(p e) -> p e",
                                                            p=64))
                                    nc.vector.reciprocal(out=r8, in_=r8)
                                    slot = rec_d[slot_id]
                                    nc.sync.dma_start(out=slot, in_=r8)
                                    recB = rp.tile([64, 512], f32, tag="recB")
                                    rbc = bass.AP(tensor=slot.tensor,
                                                  offset=slot.offset,
                                                  ap=[[0, 64]] + list(slot.ap))
                                    nc.gpsimd.dma_start(out=recB, in_=rbc)
                                    nc.vector.tensor_mul(
                                        out=outT[p0:p0 + 64, hp, qs],
                                        in0=uS[0:DH, :], in1=recB)
                    # output projection at the end (emitting it mid-kernel
                    # head-blocks the PE queue on the norm DMA chain)
                    outproj_half(0)
                    outproj_half(1)

    nc.compile()
    return nc


def make_in_maps(x, ln_g, ln_b, Wq, Wkv, Wout):
    x = np.asarray(x, np.float32)
    ln_g = np.asarray(ln_g, np.float32)
    ln_b = np.asarray(ln_b, np.float32)
    Wq = np.asarray(Wq, np.float32)
    Wkv = np.asarray(Wkv, np.float32)
    Wout = np.asarray(Wout, np.float32)

    scale = DH ** -0.5
    Wq_f = (ln_g[:, None] * Wq) * scale
    cq_f = (ln_b @ Wq) * scale
    Wk_f = ln_g[:, None] * Wkv[:, :D]
    ck_f = ln_b @ Wkv[:, :D]
    Wv_f = ln_g[:, None] * Wkv[:, D:]
    cv_f = ln_b @ Wkv[:, D:]

    bf = ml_dtypes.bfloat16
    in_maps = []
    for c in range(N_CORES):
        cols = slice((c % 4) * CW, (c % 4 + 1) * CW)
        in_maps.append({
            "x": np.ascontiguousarray(x[c // 4]).astype(bf),
            "wq": np.ascontiguousarray(Wq_f[:, cols]).astype(bf),
            "wk": np.ascontiguousarray(Wk_f[:, cols]).astype(bf),
            "wv": np.ascontiguousarray(Wv_f[:, cols]).astype(bf),
            "wo": np.ascontiguousarray(Wout[cols, :]).astype(bf),
            "cq": np.ascontiguousarray(cq_f[cols]),
            "ck": np.ascontiguousarray(ck_f[cols]),
            "cv": np.ascontiguousarray(cv_f[cols]),
            "ident": np.eye(128, dtype=bf),
        })
    return in_maps


def kernel(x, ln_g, ln_b, Wq, Wkv, Wout):
    global _compiled, LAST_RESULT
    if _compiled is None:
        _compiled = _build()
    nc = _compiled

    in_maps = make_in_maps(x, ln_g, ln_b, Wq, Wkv, Wout)
    res = run_bass_kernel_spmd(nc, in_maps, list(range(N_CORES)), trace=TRACE)
    LAST_RESULT = res

    out = np.zeros((B, N, D), np.float32)
    for c in range(N_CORES):
        out[c // 4] += res.results[c]["out"]
    return out


# revision 28
# speedup vs baseline: 1.2732x; 1.2732x over previous
"""Trainium2 Bass kernel for pre-LN multi-head attention.

Reference computation (B=2, N=2048, D=1024, H=16, DH=64):
    xn = LayerNorm(x) * g + b
    q = xn @ Wq ; k, v = split(xn @ Wkv)
    out = softmax(q k^T / sqrt(DH)) v  (per head)
    return out @ Wout
Sharding: core c handles batch b = c // 4 and heads 4*(c%4) .. 4*(c%4)+3.
Each core computes a partial output; the host sums 4 partials per batch.

Key performance structure (v2):
- x and all weights ship from the host in bf16 (halves input DMA).
- Attention processes heads in PAIRS that share the j index but differ in
  base partition (0 vs 64). The two heads' score matmuls (contraction
  DH=64) land on different PE row-groups and run CONCURRENTLY via the
  tile_position auto-derivation (row tiling), halving score cost.
- Softmax exp is split across engines: head A of each pair uses the exact
  Scalar-engine Exp; head B uses a fast exp on the Vector engine
  (Schraudolph: I = trunc(a*s + b) as int16, bitcast to bf16). The
  ~1.8% multiplicative noise on B-tiles is tolerable (rel gate 2e-2) and
  partially cancels via the shared rowsum normalization.
- Attention loop nest is (head-pair, qc, kt) with qc outer so the score
  PSUM tiles are [128, 512]: 2 heads x 2 buffers (4 banks) + 2
  accumulator banks [65, 512] fit the 8 PSUM banks with double buffering.
- Rowsum comes from a ones-column appended to V; reciprocal is
  partition-broadcast via a DRAM bounce.
"""

from contextlib import ExitStack

import numpy as np

import concourse.bass as bass
import concourse.mybir as mybir
import concourse.tile as tile
from concourse import bacc
from concourse.bass_utils import run_bass_kernel_spmd

import ml_dtypes

B, N, D = 2, 2048, 1024
H, DH = 16, 64
EPS = 1e-5
N_CORES = 8
HPC = 4          # heads per core
CW = HPC * DH    # 256 columns of q/k/v per core

f32 = mybir.dt.float32
bf16 = mybir.dt.bfloat16
i16 = mybir.dt.int16
AF = mybir.ActivationFunctionType
ALU = mybir.AluOpType

# fast-exp constants: I = trunc(A_EXP * s + B_EXP) as int16 -> bf16 bits
A_EXP = 128.0 * 1.4426950408889634     # 128 * log2(e)
B_EXP = 128.0 * (127.0 - 0.0434609) + 0.5

TRACE = False
LAST_RESULT = None
_compiled = None


def _build():
    nc = bacc.Bacc("TRN2", target_bir_lowering=False, debug=False,
                   num_devices=N_CORES)

    x_d = nc.dram_tensor("x", [N, D], bf16, kind="ExternalInput")
    wq_d = nc.dram_tensor("wq", [D, CW], bf16, kind="ExternalInput")
    wk_d = nc.dram_tensor("wk", [D, CW], bf16, kind="ExternalInput")
    wv_d = nc.dram_tensor("wv", [D, CW], bf16, kind="ExternalInput")
    wo_d = nc.dram_tensor("wo", [CW, D], bf16, kind="ExternalInput")
    cq_d = nc.dram_tensor("cq", [CW], f32, kind="ExternalInput")
    ck_d = nc.dram_tensor("ck", [CW], f32, kind="ExternalInput")
    cv_d = nc.dram_tensor("cv", [CW], f32, kind="ExternalInput")
    out_d = nc.dram_tensor("out", [N, D], f32, kind="ExternalOutput")
    rec_d = nc.dram_tensor("rec_scratch", [16, 512], f32)
    sum_d = nc.dram_tensor("sum_scratch", [16, 512], f32)
    ident_d = nc.dram_tensor("ident", [128, 128], bf16, kind="ExternalInput")

    with tile.TileContext(nc) as tc, ExitStack() as ctx:
        consts = ctx.enter_context(tc.tile_pool(name="consts", bufs=1))
        zTp = ctx.enter_context(tc.tile_pool(name="zTp", bufs=1))

        ident = consts.tile([128, 128], bf16)
        nc.sync.dma_start(out=ident, in_=ident_d[:])

        eps_t = consts.tile([128, 1], f32)
        nc.vector.memset(eps_t, EPS)

        # x prefetch first so its DMAs lead the queue; weights follow.
        # (stays open for the whole kernel; SBUF budget allows it)
        xpre = ctx.enter_context(tc.tile_pool(name="xpre", bufs=8))
        x_r = x_d.rearrange("(t u p) d -> t p u d", p=128, u=2)
        x_tiles = []
        for rt in range(8):
            xt = xpre.tile([128, 2, D], bf16, tag="xt")
            nc.sync.dma_start(out=xt, in_=x_r[rt])
            x_tiles.append(xt)

        # weights arrive bf16 — direct DMA, no staging cast
        wq_t = consts.tile([128, 8, CW], bf16)
        wk_t = consts.tile([128, 8, CW], bf16)
        wv_t = consts.tile([128, 8, CW], bf16)
        wo_t = consts.tile([128, 2, D], bf16)
        for dram, dst, spec in ((wq_d, wq_t, "(t p) m -> p t m"),
                               (wk_d, wk_t, "(t p) m -> p t m"),
                               (wv_d, wv_t, "(t p) m -> p t m"),
                               (wo_d, wo_t, "(j p) d -> p j d")):
            nc.sync.dma_start(out=dst, in_=dram.rearrange(spec, p=128))

        cq_t = consts.tile([128, 2], f32)
        nc.sync.dma_start(out=cq_t, in_=cq_d.rearrange("(j p) -> p j", p=128))
        ck_t = consts.tile([128, 2], f32)
        nc.sync.dma_start(out=ck_t, in_=ck_d.rearrange("(j p) -> p j", p=128))
        cv_t = consts.tile([128, 2], f32)
        nc.sync.dma_start(out=cv_t, in_=cv_d.rearrange("(j p) -> p j", p=128))

        # z^T in bf16: [chan(128 per tile) x 8 chan-tiles x N rows]
        zT = zTp.tile([128, 8, N], bf16)

        # dummy matmuls during the input-DMA wait: trips the HAM activity
        # window so the real matmul stream starts at the full 2.4 GHz clock
        with tc.tile_pool(name="psW", bufs=1, space="PSUM") as psW:
            pw = psW.tile([128, 128], f32)
            for i in range(40):
                nc.tensor.matmul(pw, ident, ident,
                                 start=(i == 0), stop=(i == 39))

        # ---- stage 1 + 2 interleaved: LayerNorm/transpose into zT, with
        # QKV chunk-pairs emitted as soon as their zT columns are ready so
        # the PE chews on projections while DVE/ACT run LN for later rows.
        with tc.tile_pool(name="qkT", bufs=1) as qkTp, \
             tc.tile_pool(name="vaug", bufs=1) as vaugp:
            qT = qkTp.tile([128, 2, N], bf16, tag="qT")
            kT = qkTp.tile([128, 2, N], bf16, tag="kT")
            vT = qkTp.tile([128, 2, N], bf16, tag="vT")
            vA = vaugp.tile([128, 16, HPC * (DH + 1)], bf16)
            vA4 = vA.rearrange("p k (h c) -> p k h c", h=HPC)
            ones64 = vaugp.tile([128, 64], f32)
            nc.vector.memset(ones64, 1.0)
            nc.vector.tensor_copy(
                out=vA4[:, :, :, DH:DH + 1],
                in_=ones64.rearrange("p (k h w) -> p k h w", k=16, h=HPC, w=1))

            with tc.tile_pool(name="zp", bufs=4) as zp, \
                 tc.tile_pool(name="stp", bufs=12) as stp, \
                 tc.tile_pool(name="ps1", bufs=2, space="PSUM") as ps1, \
                 tc.tile_pool(name="ps2", bufs=2, space="PSUM") as ps2:

                def ln_rt(rt):
                    xt = x_tiles[rt]
                    zt = zp.tile([128, 2, D], bf16)
                    scales = []
                    for u in range(2):
                        st = stp.tile([128, 2, 6], f32, name=f"st{rt}_{u}")
                        nc.vector.bn_stats(out=st[:, 0], in_=xt[:, u, 0:512])
                        nc.vector.bn_stats(out=st[:, 1],
                                           in_=xt[:, u, 512:1024])
                        mv = stp.tile([128, 2], f32, name=f"mv{rt}_{u}")
                        nc.vector.bn_aggr(out=mv, in_=st)
                        rstd = stp.tile([128, 1], f32, name=f"rstd{rt}_{u}")
                        nc.scalar.activation(out=rstd, in_=mv[:, 1:2],
                                             func=AF.Sqrt, bias=eps_t,
                                             scale=1.0)
                        nc.vector.reciprocal(out=rstd, in_=rstd)
                        nmr = stp.tile([128, 1], f32, name=f"nmr{rt}_{u}")
                        nc.vector.tensor_scalar(out=nmr, in0=mv[:, 0:1],
                                                scalar1=rstd, scalar2=-1.0,
                                                op0=ALU.mult, op1=ALU.mult)
                        scales.append((rstd, nmr))
                    for u in range(2):
                        rstd, nmr = scales[u]
                        for half in range(2):
                            hs = slice(half * 512, (half + 1) * 512)
                            nc.scalar.activation(out=zt[:, u, hs],
                                                 in_=xt[:, u, hs],
                                                 func=AF.Identity, bias=nmr,
                                                 scale=rstd)
                        r0 = rt * 256 + u * 128
                        pt = ps1.tile([128, 1024], bf16)
                        for cc in range(8):
                            nc.tensor.transpose(
                                pt[:, cc * 128:(cc + 1) * 128],
                                zt[:, u, cc * 128:(cc + 1) * 128], ident)
                        cp_out = zT[:, :, r0:r0 + 128]
                        cp_in = pt.rearrange("p (c n) -> p c n", c=8)
                        if u == 0:
                            nc.vector.tensor_copy(out=cp_out, in_=cp_in)
                        else:
                            nc.scalar.activation(out=cp_out, in_=cp_in,
                                                 func=AF.Identity, bias=0.0,
                                                 scale=1.0)

                def qkv_chunkpair(cp):
                    chks = (2 * cp, 2 * cp + 1)
                    for w_t, c_t, dest in ((wq_t, cq_t, qT), (wk_t, ck_t, kT),
                                           (wv_t, cv_t, vT)):
                        for j in range(2):
                            pqs = {c: ps2.tile([128, 512], f32,
                                               tag=f"pq{c % 2}",
                                               name=f"pq{cp}_{c}")
                                   for c in chks}
                            for t in range(8):
                                lhs = w_t[:, t, j * 128:(j + 1) * 128]
                                for chk in chks:
                                    nc.tensor.matmul(
                                        pqs[chk], lhs,
                                        zT[:, t, chk * 512:(chk + 1) * 512],
                                        start=(t == 0), stop=(t == 7))
                            for chk in chks:
                                ns = slice(chk * 512, (chk + 1) * 512)
                                nc.vector.tensor_scalar_add(
                                    out=dest[:, j, ns], in0=pqs[chk],
                                    scalar1=c_t[:, j:j + 1])

                ln_rt(0), ln_rt(1), ln_rt(2), ln_rt(3)
                qkv_chunkpair(0)
                ln_rt(4), ln_rt(5), ln_rt(6), ln_rt(7)
                qkv_chunkpair(1)

                # transpose vT -> vA[keys, c] (heads 2j, 2j+1 per j-half)
                with tc.tile_pool(name="psVT", bufs=1, space="PSUM") as psVT:
                    for kt in range(16):
                        ks = slice(kt * 128, (kt + 1) * 128)
                        for j in range(2):
                            pt = psVT.tile([128, 128], bf16, tag=f"vt{j}",
                                           name=f"vt{kt}_{j}")
                            nc.tensor.transpose(pt, vT[:, j, ks], ident)
                            nc.vector.tensor_copy(
                                out=vA4[:, kt, 2 * j:2 * j + 2, 0:DH],
                                in_=pt.rearrange("p (h c) -> p h c", h=2))

            # ---- stage 3: attention, head pairs x qc pairs ----
            # Per (hp, qcp, kt): the two heads' score MMs go to PE row-groups
            # 0/64 and run concurrently; each lhsT serves the two qc chunks.
            # exp is split: ACT does head A + 1/4 of head B (exact), DVE the
            # other 3/4 of head B (fast int16-bitcast exp).
            with tc.tile_pool(name="oT", bufs=1) as oTp:
                outT = oTp.tile([128, 2, N], bf16)
                with tc.tile_pool(name="Ep", bufs=4) as Ep, \
                     tc.tile_pool(name="rp", bufs=6) as rp, \
                     tc.tile_pool(name="osb", bufs=4) as osb, \
                     tc.tile_pool(name="psD", bufs=1, space="PSUM") as psD, \
                     tc.tile_pool(name="psU", bufs=1, space="PSUM") as psU:
                    out_r = out_d.rearrange("(m p) d -> m p d", p=128)

                    def outproj_half(qcp):
                        # m-tiles 8*qcp .. 8*qcp+7 use outT cols that are
                        # final once both head-pairs finished this qcp;
                        # PSUM comes from the psD ring (same tile shape)
                        for m in range(8 * qcp, 8 * qcp + 8):
                            pOs = [psD.tile([128, 512], f32,
                                            tag=f"pD{i}_0",
                                            name=f"pO{m}_{i}")
                                   for i in range(2)]
                            for j in range(2):
                                lhs = outT[:, j, m * 128:(m + 1) * 128]
                                for nn in range(2):
                                    nc.tensor.matmul(
                                        pOs[nn], lhs,
                                        wo_t[:, j, nn * 512:(nn + 1) * 512],
                                        start=(j == 0), stop=(j == 1))
                            for nn in range(2):
                                ot = osb.tile([128, 512], f32)
                                if nn == 0:
                                    nc.vector.tensor_copy(out=ot, in_=pOs[nn])
                                else:
                                    nc.scalar.activation(out=ot, in_=pOs[nn],
                                                         func=AF.Identity,
                                                         bias=0.0, scale=1.0)
                                nc.sync.dma_start(
                                    out=out_r[m][:, nn * 512:(nn + 1) * 512],
                                    in_=ot)

                    for qcp in range(2):         # query column pair (2x512)
                        for hp in range(2):      # head pair: heads 2hp, 2hp+1
                            pUs = [[psU.tile([DH + 1, 512], f32,
                                             tag=f"pU{hi}_{q2}",
                                             name=f"pU{hp}_{qcp}_{hi}_{q2}")
                                    for q2 in range(2)] for hi in range(2)]
                            Ets = {}

                            def scores_exp(kt, hp=hp, qcp=qcp, pUs=pUs,
                                           Ets=Ets):
                                ks = slice(kt * 128, (kt + 1) * 128)
                                pDs = [[psD.tile([128, 512], f32,
                                                 tag=f"pD{hi}_{q2}",
                                                 name=f"pD{hp}{qcp}{kt}{hi}{q2}")
                                        for q2 in range(2)]
                                       for hi in range(2)]
                                # A0 A1 then B0 B1: each kT lhsT serves two
                                # MMs; B's row-group differs so it overlaps A
                                for hi in range(2):
                                    p0 = 64 * hi
                                    for q2 in range(2):
                                        qs = slice((qcp * 2 + q2) * 512,
                                                   (qcp * 2 + q2 + 1) * 512)
                                        nc.tensor.matmul(
                                            pDs[hi][q2],
                                            kT[p0:p0 + 64, hp, ks],
                                            qT[p0:p0 + 64, hp, qs],
                                            start=True, stop=True)
                                # exp: one whole-tile op per pD tag so each
                                # tag frees after a single engine pass —
                                # head A on ACT (exact), head B on DVE (fast)
                                EtA = Ep.tile([128, 2, 512], bf16, tag="EtA")
                                EtB = Ep.tile([128, 2, 512], i16, tag="EtB")
                                EtBf = EtB.bitcast(bf16)
                                for q2 in range(2):
                                    nc.scalar.activation(
                                        out=EtA[:, q2], in_=pDs[0][q2],
                                        func=AF.Exp, bias=0.0, scale=1.0)
                                    nc.vector.tensor_scalar(
                                        out=EtB[:, q2], in0=pDs[1][q2],
                                        scalar1=A_EXP, scalar2=B_EXP,
                                        op0=ALU.mult, op1=ALU.add)
                                Ets[kt] = (EtA, EtBf)

                            def ev(kt, hp=hp, pUs=pUs, Ets=Ets):
                                EtA, EtBf = Ets.pop(kt)
                                for hi, Et in ((0, EtA), (1, EtBf)):
                                    h = 2 * hp + hi
                                    vlhs = vA[:, kt,
                                              h * (DH + 1):(h + 1) * (DH + 1)]
                                    for q2 in range(2):
                                        nc.tensor.matmul(
                                            pUs[hi][q2][0:DH + 1, :],
                                            vlhs, Et[:, q2],
                                            start=(kt == 0), stop=(kt == 15))

                            # software-pipelined emission: EV trails the
                            # scores/exp of the NEXT kt so the in-order PE
                            # queue never head-blocks on exp or on the
                            # previous block's normalization
                            scores_exp(0)
                            scores_exp(1)
                            for kt in range(2, 16):
                                ev(kt - 2)
                                scores_exp(kt)
                            ev(14)
                            ev(15)
                            # normalization: rowsum -> DRAM bounce -> recip
                            for hi in range(2):
                                for q2 in range(2):
                                    pU = pUs[hi][q2]
                                    qc = qcp * 2 + q2
                                    qs = slice(qc * 512, (qc + 1) * 512)
                                    p0 = 64 * hi
                                    slot_id = hp * 8 + qc * 2 + hi
                                    # full copy frees the PSUM accumulator
                                    # immediately; norm runs off the copy
                                    uS = rp.tile([128, 512], f32, tag="uS",
                                                 name=f"uS{slot_id}")
                                    nc.vector.tensor_copy(
                                        out=uS[0:DH + 1, :],
                                        in_=pU[0:DH + 1, :])
                                    sslot = sum_d[slot_id]
                                    nc.sync.dma_start(out=sslot,
                                                      in_=uS[DH:DH + 1, :])
                                    r8 = rp.tile([64, 8], f32, tag="r8",
                                                 name=f"r8{slot_id}")
                                    nc.sync.dma_start(
                                        out=r8,
                                        in_=sslot.rearrange("(p e) -> p e",
                                                            p=64))
                                    nc.vector.reciprocal(out=r8, in_=r8)
                                    slot = rec_d[slot_id]
                                    nc.sync.dma_start(out=slot, in_=r8)
                                    recB = rp.tile([64, 512], f32, tag="recB")
                                    rbc = bass.AP(tensor=slot.tensor,
                                                  offset=slot.offset,
                                                  ap=[[0, 64]] + list(slot.ap))
                                    nc.gpsimd.dma_start(out=recB, in_=rbc)
                                    nc.vector.tensor_mul(
                                        out=outT[p0:p0 + 64, hp, qs],
                                        in0=uS[0:DH, :], in1=recB)
                    # output projection at the end (emitting it mid-kernel
                    # head-blocks the PE queue on the norm DMA chain)
                    outproj_half(0)
                    outproj_half(1)

    nc.compile()
    return nc


def make_in_maps(x, ln_g, ln_b, Wq, Wkv, Wout):
    x = np.asarray(x, np.float32)
    ln_g = np.asarray(ln_g, np.float32)
    ln_b = np.asarray(ln_b, np.float32)
    Wq = np.asarray(Wq, np.float32)
    Wkv = np.asarray(Wkv, np.float32)
    Wout = np.asarray(Wout, np.float32)

    scale = DH ** -0.5
    Wq_f = (ln_g[:, None] * Wq) * scale
    cq_f = (ln_b @ Wq) * scale
    Wk_f = ln_g[:, None] * Wkv[:, :D]
    ck_f = ln_b @ Wkv[:, :D]
    Wv_f = ln_g[:, None] * Wkv[:, D:]
    cv_f = ln_b @ Wkv[:, D:]

    bf = ml_dtypes.bfloat16
    in_maps = []
    for c in range(N_CORES):
        cols = slice((c % 4) * CW, (c % 4 + 1) * CW)
        in_maps.append({
            "x": np.ascontiguousarray(x[c // 4]).astype(bf),
            "wq": np.ascontiguousarray(Wq_f[:, cols]).astype(bf),
            "wk": np.ascontiguousarray(Wk_f[:, cols]).astype(bf),
            "wv": np.ascontiguousarray(Wv_f[:, cols]).astype(bf),
            "wo": np.ascontiguousarray(Wout[cols, :]).astype(bf),
            "cq": np.ascontiguousarray(cq_f[cols]),
            "ck": np.ascontiguousarray(ck_f[cols]),
            "cv": np.ascontiguousarray(cv_f[cols]),
            "ident": np.eye(128, dtype=bf),
        })
    return in_maps


def kernel(x, ln_g, ln_b, Wq, Wkv, Wout):
    global _compiled, LAST_RESULT
    if _compiled is None:
        _compiled = _build()
    nc = _compiled

    in_maps = make_in_maps(x, ln_g, ln_b, Wq, Wkv, Wout)
    res = run_bass_kernel_spmd(nc, in_maps, list(range(N_CORES)), trace=TRACE)
    LAST_RESULT = res

    out = np.zeros((B, N, D), np.float32)
    for c in range(N_CORES):
        out[c // 4] += res.results[c]["out"]
    return out


# revision 32
# speedup vs baseline: 1.2737x; 1.0004x over previous
"""Trainium2 Bass kernel for pre-LN multi-head attention.

Reference computation (B=2, N=2048, D=1024, H=16, DH=64):
    xn = LayerNorm(x) * g + b
    q = xn @ Wq ; k, v = split(xn @ Wkv)
    out = softmax(q k^T / sqrt(DH)) v  (per head)
    return out @ Wout
Sharding: core c handles batch b = c // 4 and heads 4*(c%4) .. 4*(c%4)+3.
Each core computes a partial output; the host sums 4 partials per batch.

Measured ~249 us HW exec (baseline 315 us). Key structure:
- x and all weights ship from the host in bf16 (halves input DMA).
- Warm-up matmuls on the identity run during the input-DMA wait to trip
  the HAM activity window so real matmuls start at 2.4 GHz.
- Stage 1 (LN + PE-transpose into zT) is interleaved with QKV chunk-pair
  emission so the PE chews projections while DVE/ACT normalize later rows.
- q, k AND v are all weight-stationary projections (lhsT held across two
  512-col moving chunks); v comes out transposed (vT [c, n]) and is
  PE-transposed into vA [keys, c] (+ ones column for the rowsum).
- Attention nest: (qcp, head-pair, kt). The two heads share the j index
  but differ in base partition (0/64): their score matmuls land on
  different PE row-groups and overlap; each kT/vA lhsT serves two 512-col
  moving chunks (1:2 reuse). PSUM: 4x pD [128,512] + 4x pU [65,512].
- Emission is software-pipelined: EV trails scores/exp by two kt so the
  in-order PE queue never head-blocks on exp latency or on the previous
  block's normalization.
- Softmax exp is split per head: head A exact on ScalarE, head B fast-exp
  on VectorE (Schraudolph: I = trunc(a*s + b) as int16, bitcast to bf16;
  ~1.8% weight noise, partially cancelled by the shared rowsum
  normalization; end-to-end rel err ~1.04e-2 vs the 2e-2 gate).
- pU is copied whole to SBUF right after the last EV so the accumulator
  bank frees immediately; the rowsum/reciprocal/broadcast chain (DRAM
  bounce) runs off the copy, fully hidden under the next block.
"""

from contextlib import ExitStack

import numpy as np

import concourse.bass as bass
import concourse.mybir as mybir
import concourse.tile as tile
from concourse import bacc
from concourse.bass_utils import run_bass_kernel_spmd

import ml_dtypes

B, N, D = 2, 2048, 1024
H, DH = 16, 64
EPS = 1e-5
N_CORES = 8
HPC = 4          # heads per core
CW = HPC * DH    # 256 columns of q/k/v per core

f32 = mybir.dt.float32
bf16 = mybir.dt.bfloat16
i16 = mybir.dt.int16
AF = mybir.ActivationFunctionType
ALU = mybir.AluOpType

# fast-exp constants: I = trunc(A_EXP * s + B_EXP) as int16 -> bf16 bits
A_EXP = 128.0 * 1.4426950408889634     # 128 * log2(e)
B_EXP = 128.0 * (127.0 - 0.0434609) + 0.5

TRACE = False
LAST_RESULT = None
_compiled = None


def _build():
    nc = bacc.Bacc("TRN2", target_bir_lowering=False, debug=False,
                   num_devices=N_CORES)

    x_d = nc.dram_tensor("x", [N, D], bf16, kind="ExternalInput")
    wq_d = nc.dram_tensor("wq", [D, CW], bf16, kind="ExternalInput")
    wk_d = nc.dram_tensor("wk", [D, CW], bf16, kind="ExternalInput")
    wv_d = nc.dram_tensor("wv", [D, CW], bf16, kind="ExternalInput")
    wo_d = nc.dram_tensor("wo", [CW, D], bf16, kind="ExternalInput")
    cq_d = nc.dram_tensor("cq", [CW], f32, kind="ExternalInput")
    ck_d = nc.dram_tensor("ck", [CW], f32, kind="ExternalInput")
    cv_d = nc.dram_tensor("cv", [CW], f32, kind="ExternalInput")
    out_d = nc.dram_tensor("out", [N, D], bf16, kind="ExternalOutput")
    rec_d = nc.dram_tensor("rec_scratch", [16, 512], f32)
    sum_d = nc.dram_tensor("sum_scratch", [16, 512], f32)
    ident_d = nc.dram_tensor("ident", [128, 128], bf16, kind="ExternalInput")

    with tile.TileContext(nc) as tc, ExitStack() as ctx:
        consts = ctx.enter_context(tc.tile_pool(name="consts", bufs=1))
        zTp = ctx.enter_context(tc.tile_pool(name="zTp", bufs=1))

        ident = consts.tile([128, 128], bf16)
        nc.sync.dma_start(out=ident, in_=ident_d[:])

        eps_t = consts.tile([128, 1], f32)
        nc.vector.memset(eps_t, EPS)

        # x prefetch first so its DMAs lead the queue; weights follow.
        # (stays open for the whole kernel; SBUF budget allows it)
        xpre = ctx.enter_context(tc.tile_pool(name="xpre", bufs=8))
        x_r = x_d.rearrange("(t u p) d -> t p u d", p=128, u=2)
        x_tiles = []
        for rt in range(8):
            xt = xpre.tile([128, 2, D], bf16, tag="xt")
            nc.sync.dma_start(out=xt, in_=x_r[rt])
            x_tiles.append(xt)

        # weights arrive bf16 — direct DMA, no staging cast
        wq_t = consts.tile([128, 8, CW], bf16)
        wk_t = consts.tile([128, 8, CW], bf16)
        wv_t = consts.tile([128, 8, CW], bf16)
        wo_t = consts.tile([128, 2, D], bf16)
        for dram, dst, spec in ((wq_d, wq_t, "(t p) m -> p t m"),
                               (wk_d, wk_t, "(t p) m -> p t m"),
                               (wv_d, wv_t, "(t p) m -> p t m"),
                               (wo_d, wo_t, "(j p) d -> p j d")):
            nc.sync.dma_start(out=dst, in_=dram.rearrange(spec, p=128))

        cq_t = consts.tile([128, 2], f32)
        nc.sync.dma_start(out=cq_t, in_=cq_d.rearrange("(j p) -> p j", p=128))
        ck_t = consts.tile([128, 2], f32)
        nc.sync.dma_start(out=ck_t, in_=ck_d.rearrange("(j p) -> p j", p=128))
        cv_t = consts.tile([128, 2], f32)
        nc.sync.dma_start(out=cv_t, in_=cv_d.rearrange("(j p) -> p j", p=128))

        # z^T in bf16: [chan(128 per tile) x 8 chan-tiles x N rows]
        zT = zTp.tile([128, 8, N], bf16)

        # dummy matmuls during the input-DMA wait: trips the HAM activity
        # window so the real matmul stream starts at the full 2.4 GHz clock
        with tc.tile_pool(name="psW", bufs=1, space="PSUM") as psW:
            pw = psW.tile([128, 128], f32)
            for i in range(40):
                nc.tensor.matmul(pw, ident, ident,
                                 start=(i == 0), stop=(i == 39))

        # ---- stage 1 + 2 interleaved: LayerNorm/transpose into zT, with
        # QKV chunk-pairs emitted as soon as their zT columns are ready so
        # the PE chews on projections while DVE/ACT run LN for later rows.
        with tc.tile_pool(name="qkT", bufs=1) as qkTp, \
             tc.tile_pool(name="vaug", bufs=1) as vaugp:
            qT = qkTp.tile([128, 2, N], bf16, tag="qT")
            kT = qkTp.tile([128, 2, N], bf16, tag="kT")
            vT = qkTp.tile([128, 2, N], bf16, tag="vT")
            vA = vaugp.tile([128, 16, HPC * (DH + 1)], bf16)
            vA4 = vA.rearrange("p k (h c) -> p k h c", h=HPC)
            ones64 = vaugp.tile([128, 64], f32)
            nc.vector.memset(ones64, 1.0)
            nc.vector.tensor_copy(
                out=vA4[:, :, :, DH:DH + 1],
                in_=ones64.rearrange("p (k h w) -> p k h w", k=16, h=HPC, w=1))

            with tc.tile_pool(name="zp", bufs=4) as zp, \
                 tc.tile_pool(name="stp", bufs=12) as stp, \
                 tc.tile_pool(name="ps1", bufs=2, space="PSUM") as ps1, \
                 tc.tile_pool(name="ps2", bufs=2, space="PSUM") as ps2:

                def ln_rt(rt):
                    xt = x_tiles[rt]
                    zt = zp.tile([128, 2, D], bf16)
                    scales = []
                    for u in range(2):
                        st = stp.tile([128, 2, 6], f32, name=f"st{rt}_{u}")
                        nc.vector.bn_stats(out=st[:, 0], in_=xt[:, u, 0:512])
                        nc.vector.bn_stats(out=st[:, 1],
                                           in_=xt[:, u, 512:1024])
                        mv = stp.tile([128, 2], f32, name=f"mv{rt}_{u}")
                        nc.vector.bn_aggr(out=mv, in_=st)
                        rstd = stp.tile([128, 1], f32, name=f"rstd{rt}_{u}")
                        nc.scalar.activation(out=rstd, in_=mv[:, 1:2],
                                             func=AF.Sqrt, bias=eps_t,
                                             scale=1.0)
                        nc.vector.reciprocal(out=rstd, in_=rstd)
                        nmr = stp.tile([128, 1], f32, name=f"nmr{rt}_{u}")
                        nc.vector.tensor_scalar(out=nmr, in0=mv[:, 0:1],
                                                scalar1=rstd, scalar2=-1.0,
                                                op0=ALU.mult, op1=ALU.mult)
                        scales.append((rstd, nmr))
                    for u in range(2):
                        rstd, nmr = scales[u]
                        for half in range(2):
                            hs = slice(half * 512, (half + 1) * 512)
                            nc.scalar.activation(out=zt[:, u, hs],
                                                 in_=xt[:, u, hs],
                                                 func=AF.Identity, bias=nmr,
                                                 scale=rstd)
                        r0 = rt * 256 + u * 128
                        pt = ps1.tile([128, 1024], bf16)
                        for cc in range(8):
                            nc.tensor.transpose(
                                pt[:, cc * 128:(cc + 1) * 128],
                                zt[:, u, cc * 128:(cc + 1) * 128], ident)
                        cp_out = zT[:, :, r0:r0 + 128]
                        cp_in = pt.rearrange("p (c n) -> p c n", c=8)
                        if u == 0:
                            nc.vector.tensor_copy(out=cp_out, in_=cp_in)
                        else:
                            nc.scalar.activation(out=cp_out, in_=cp_in,
                                                 func=AF.Identity, bias=0.0,
                                                 scale=1.0)

                def qkv_chunkpair(cp):
                    chks = (2 * cp, 2 * cp + 1)
                    for w_t, c_t, dest in ((wq_t, cq_t, qT), (wk_t, ck_t, kT),
                                           (wv_t, cv_t, vT)):
                        for j in range(2):
                            pqs = {c: ps2.tile([128, 512], f32,
                                               tag=f"pq{c % 2}",
                                               name=f"pq{cp}_{c}")
                                   for c in chks}
                            for t in range(8):
                                lhs = w_t[:, t, j * 128:(j + 1) * 128]
                                for chk in chks:
                                    nc.tensor.matmul(
                                        pqs[chk], lhs,
                                        zT[:, t, chk * 512:(chk + 1) * 512],
                                        start=(t == 0), stop=(t == 7))
                            for chk in chks:
                                ns = slice(chk * 512, (chk + 1) * 512)
                                nc.vector.tensor_scalar_add(
                                    out=dest[:, j, ns], in0=pqs[chk],
                                    scalar1=c_t[:, j:j + 1])

                ln_rt(0), ln_rt(1), ln_rt(2), ln_rt(3)
                qkv_chunkpair(0)
                ln_rt(4), ln_rt(5), ln_rt(6), ln_rt(7)
                qkv_chunkpair(1)

                # transpose vT -> vA[keys, c] (heads 2j, 2j+1 per j-half)
                with tc.tile_pool(name="psVT", bufs=1, space="PSUM") as psVT:
                    for kt in range(16):
                        ks = slice(kt * 128, (kt + 1) * 128)
                        for j in range(2):
                            pt = psVT.tile([128, 128], bf16, tag=f"vt{j}",
                                           name=f"vt{kt}_{j}")
                            nc.tensor.transpose(pt, vT[:, j, ks], ident)
                            nc.vector.tensor_copy(
                                out=vA4[:, kt, 2 * j:2 * j + 2, 0:DH],
                                in_=pt.rearrange("p (h c) -> p h c", h=2))

            # ---- stage 3: attention, head pairs x qc pairs ----
            # Per (hp, qcp, kt): the two heads' score MMs go to PE row-groups
            # 0/64 and run concurrently; each lhsT serves the two qc chunks.
            # exp is split: ACT does head A + 1/4 of head B (exact), DVE the
            # other 3/4 of head B (fast int16-bitcast exp).
            with tc.tile_pool(name="oT", bufs=1) as oTp:
                outT = oTp.tile([128, 2, N], bf16)
                with tc.tile_pool(name="Ep", bufs=4) as Ep, \
                     tc.tile_pool(name="rp", bufs=6) as rp, \
                     tc.tile_pool(name="osb", bufs=4) as osb, \
                     tc.tile_pool(name="psD", bufs=1, space="PSUM") as psD, \
                     tc.tile_pool(name="psU", bufs=1, space="PSUM") as psU:
                    out_r = out_d.rearrange("(m p) d -> m p d", p=128)

                    def outproj_half(qcp):
                        # m-tiles 8*qcp .. 8*qcp+7 use outT cols that are
                        # final once both head-pairs finished this qcp;
                        # PSUM comes from the psD ring (same tile shape)
                        for m in range(8 * qcp, 8 * qcp + 8):
                            pOs = [psD.tile([128, 512], f32,
                                            tag=f"pD{i}_0",
                                            name=f"pO{m}_{i}")
                                   for i in range(2)]
                            for j in range(2):
                                lhs = outT[:, j, m * 128:(m + 1) * 128]
                                for nn in range(2):
                                    nc.tensor.matmul(
                                        pOs[nn], lhs,
                                        wo_t[:, j, nn * 512:(nn + 1) * 512],
                                        start=(j == 0), stop=(j == 1))
                            for nn in range(2):
                                ot = osb.tile([128, 512], bf16)
                                if nn == 0:
                                    nc.vector.tensor_copy(out=ot, in_=pOs[nn])
                                else:
                                    nc.scalar.activation(out=ot, in_=pOs[nn],
                                                         func=AF.Identity,
                                                         bias=0.0, scale=1.0)
                                nc.sync.dma_start(
                                    out=out_r[m][:, nn * 512:(nn + 1) * 512],
                                    in_=ot)

                    for qcp in range(2):         # query column pair (2x512)
                        for hp in range(2):      # head pair: heads 2hp, 2hp+1
                            pUs = [[psU.tile([DH + 1, 512], f32,
                                             tag=f"pU{hi}_{q2}",
                                             name=f"pU{hp}_{qcp}_{hi}_{q2}")
                                    for q2 in range(2)] for hi in range(2)]
                            Ets = {}

                            def scores_exp(kt, hp=hp, qcp=qcp, pUs=pUs,
                                           Ets=Ets):
                                ks = slice(kt * 128, (kt + 1) * 128)
                                pDs = [[psD.tile([128, 512], f32,
                                                 tag=f"pD{hi}_{q2}",
                                                 name=f"pD{hp}{qcp}{kt}{hi}{q2}")
                                        for q2 in range(2)]
                                       for hi in range(2)]
                                # A0 A1 then B0 B1: each kT lhsT serves two
                                # MMs; B's row-group differs so it overlaps A
                                for hi in range(2):
                                    p0 = 64 * hi
                                    for q2 in range(2):
                                        qs = slice((qcp * 2 + q2) * 512,
                                                   (qcp * 2 + q2 + 1) * 512)
                                        nc.tensor.matmul(
                                            pDs[hi][q2],
                                            kT[p0:p0 + 64, hp, ks],
                                            qT[p0:p0 + 64, hp, qs],
                                            start=True, stop=True)
                                # exp: one whole-tile op per pD tag so each
                                # tag frees after a single engine pass —
                                # head A on ACT (exact), head B on DVE (fast)
                                EtA = Ep.tile([128, 2, 512], bf16, tag="EtA")
                                EtB = Ep.tile([128, 2, 512], i16, tag="EtB")
                                EtBf = EtB.bitcast(bf16)
                                for q2 in range(2):
                                    nc.scalar.activation(
                                        out=EtA[:, q2], in_=pDs[0][q2],
                                        func=AF.Exp, bias=0.0, scale=1.0)
                                    nc.vector.tensor_scalar(
                                        out=EtB[:, q2], in0=pDs[1][q2],
                                        scalar1=A_EXP, scalar2=B_EXP,
                                        op0=ALU.mult, op1=ALU.add)
                                Ets[kt] = (EtA, EtBf)

                            def ev(kt, hp=hp, pUs=pUs, Ets=Ets):
                                EtA, EtBf = Ets.pop(kt)
                                for hi, Et in ((0, EtA), (1, EtBf)):
                                    h = 2 * hp + hi
                                    vlhs = vA[:, kt,
                                              h * (DH + 1):(h + 1) * (DH + 1)]
                                    for q2 in range(2):
                                        nc.tensor.matmul(
                                            pUs[hi][q2][0:DH + 1, :],
                                            vlhs, Et[:, q2],
                                            start=(kt == 0), stop=(kt == 15))

                            # software-pipelined emission: EV trails the
                            # scores/exp of the NEXT kt so the in-order PE
                            # queue never head-blocks on exp or on the
                            # previous block's normalization
                            scores_exp(0)
                            scores_exp(1)
                            for kt in range(2, 16):
                                ev(kt - 2)
                                scores_exp(kt)
                            ev(14)
                            ev(15)
                            # normalization: rowsum -> DRAM bounce -> recip
                            for hi in range(2):
                                for q2 in range(2):
                                    pU = pUs[hi][q2]
                                    qc = qcp * 2 + q2
                                    qs = slice(qc * 512, (qc + 1) * 512)
                                    p0 = 64 * hi
                                    slot_id = hp * 8 + qc * 2 + hi
                                    # full copy frees the PSUM accumulator
                                    # immediately; norm runs off the copy
                                    uS = rp.tile([128, 512], f32, tag="uS",
                                                 name=f"uS{slot_id}")
                                    nc.vector.tensor_copy(
                                        out=uS[0:DH + 1, :],
                                        in_=pU[0:DH + 1, :])
                                    sslot = sum_d[slot_id]
                                    nc.sync.dma_start(out=sslot,
                                                      in_=uS[DH:DH + 1, :])
                                    r8 = rp.tile([64, 8], f32, tag="r8",
                                                 name=f"r8{slot_id}")
                                    nc.sync.dma_start(
                                        out=r8,
                                        in_=sslot.rearrange("(p e) -> p e",
                                                            p=64))
                                    nc.vector.reciprocal(out=r8, in_=r8)
                                    slot = rec_d[slot_id]
                                    nc.sync.dma_start(out=slot, in_=r8)
                                    recB = rp.tile([64, 512], f32, tag="recB")
                                    rbc = bass.AP(tensor=slot.tensor,
                                                  offset=slot.offset,
                                                  ap=[[0, 64]] + list(slot.ap))
                                    nc.gpsimd.dma_start(out=recB, in_=rbc)
                                    nc.vector.tensor_mul(
                                        out=outT[p0:p0 + 64, hp, qs],
                                        in0=uS[0:DH, :], in1=recB)
                    # output projection at the end (emitting it mid-kernel
                    # head-blocks the PE queue on the norm DMA chain)
                    outproj_half(0)
                    outproj_half(1)

    nc.compile()
    return nc


def make_in_maps(x, ln_g, ln_b, Wq, Wkv, Wout):
    x = np.asarray(x, np.float32)
    ln_g = np.asarray(ln_g, np.float32)
    ln_b = np.asarray(ln_b, np.float32)
    Wq = np.asarray(Wq, np.float32)
    Wkv = np.asarray(Wkv, np.float32)
    Wout = np.asarray(Wout, np.float32)

    scale = DH ** -0.5
    Wq_f = (ln_g[:, None] * Wq) * scale
    cq_f = (ln_b @ Wq) * scale
    Wk_f = ln_g[:, None] * Wkv[:, :D]
    ck_f = ln_b @ Wkv[:, :D]
    Wv_f = ln_g[:, None] * Wkv[:, D:]
    cv_f = ln_b @ Wkv[:, D:]

    bf = ml_dtypes.bfloat16
    in_maps = []
    for c in range(N_CORES):
        cols = slice((c % 4) * CW, (c % 4 + 1) * CW)
        in_maps.append({
            "x": np.ascontiguousarray(x[c // 4]).astype(bf),
            "wq": np.ascontiguousarray(Wq_f[:, cols]).astype(bf),
            "wk": np.ascontiguousarray(Wk_f[:, cols]).astype(bf),
            "wv": np.ascontiguousarray(Wv_f[:, cols]).astype(bf),
            "wo": np.ascontiguousarray(Wout[cols, :]).astype(bf),
            "cq": np.ascontiguousarray(cq_f[cols]),
            "ck": np.ascontiguousarray(ck_f[cols]),
            "cv": np.ascontiguousarray(cv_f[cols]),
            "ident": np.eye(128, dtype=bf),
        })
    return in_maps


def kernel(x, ln_g, ln_b, Wq, Wkv, Wout):
    global _compiled, LAST_RESULT
    if _compiled is None:
        _compiled = _build()
    nc = _compiled

    in_maps = make_in_maps(x, ln_g, ln_b, Wq, Wkv, Wout)
    res = run_bass_kernel_spmd(nc, in_maps, list(range(N_CORES)), trace=TRACE)
    LAST_RESULT = res

    out = np.zeros((B, N, D), np.float32)
    for c in range(N_CORES):
        out[c // 4] += np.asarray(res.results[c]["out"], np.float32)
    return out
